# revision 5
# baseline (speedup 1.0000x reference)
"""Causal self-attention (dense transformer block) on 8 Trainium2 NeuronCores.

Sharding: tensor-parallel over heads x data-parallel over batch.
  - 8 cores = 2 batch groups x 4 cores; each core owns 1 batch element and
    4 of the 16 heads (head_dim 64 -> 256 local channels, 2 "pairs" of heads).
  - Host pre-transposes x and the weight slices; everything ships as bf16
    (verified 3.5e-3 rel err vs the 2e-2 gate) so input DMA is ~6.3 MB/core.
  - Each core computes qkv, causal attention in "S^T" layout (scores[k, q],
    k on partitions), and its partial c_proj; host sums 4 partials per batch.

Math notes (carried over from the fp32r version):
  - k-bias cancels in softmax; v-bias passes through to a constant output
    offset w_proj @ b_v added on host together with b_proj.
  - Softmax skips max-subtraction: |scores/8| <~ 3 for this distribution.
  - V carries a ones column so softmax denominators fall out of the attV
    matmul (row 64 of the PSUM accumulator).

Schedule notes (what changed vs the 279us version):
  - x streams in by 512-token chunks and qkv projections start per chunk,
    so the PE is busy ~7us in instead of ~40us.
  - Scores matmuls for the two heads of a pair contract over 64 partitions
    each and are emitted back-to-back at PE row-tiles 0/64 (tile_position
    auto-derives from base partitions), so they run concurrently.
  - exp for both heads of a pair is ONE ScalarE activation over a strided
    [128, 2, w] view of the scores PSUM slab -> halves ACT instruction
    overhead (352 cyc/op). es is bf16.
  - Causal diag masking = 2 cheap DVE multiplies with a precomputed
    lower-tri bf16 mask (off GpSimd's affine_select path).
  - Attention runs chunk-major (q-chunks of 512), pairs alternating, with
    qkv projection chunks and per-chunk c_proj interleaved in emission
    order so the PE always has dense independent matmul work -> the HAM
    clock gate stays at 8/8 (baseline spent 136us at 4/8).
  - Softmax denominators: one [4, 512] reciprocal per chunk (DVE recip
    cost is free-dim-bound, so batching partitions is ~free), gpsimd
    partition_broadcast, in-place bf16 normalize of y.
  - c_proj per chunk as soon as both pairs' y are normalized; output
    streams out per 128-token tile.
"""

import numpy as np
from contextlib import ExitStack

import ml_dtypes

import concourse.bass as bass
import concourse.tile as tile
from concourse import bacc, mybir
from concourse.bass_utils import run_bass_kernel_spmd

FP32 = mybir.dt.float32
BF16 = mybir.dt.bfloat16
AF = mybir.ActivationFunctionType

B, T_FULL, C = 2, 2048, 1024
H, D = 16, 64
NCORES = 8
CPG = 4          # cores per batch group
HPC = H // CPG   # heads per core = 4
HL = HPC * D     # local channels = 256
NP = 2           # head pairs per core
CT = C // 128    # contraction tiles = 8


def build_bass(T=T_FULL):
    """Emit the SPMD Bass/Tile program for one core."""
    assert T % 512 == 0
    TT = T // 128          # 128-token t-tiles (16)
    NCH = T // 512         # 512-token chunks (4)

    nc = bacc.Bacc("TRN2", target_bir_lowering=False, debug=False,
                   num_devices=NCORES)

    xT_d = nc.dram_tensor("xT", [C, T], BF16, kind="ExternalInput")
    wqkvT_d = nc.dram_tensor("wqkvT", [C, 3 * HL], BF16, kind="ExternalInput")
    bq_d = nc.dram_tensor("bq", [HL], FP32, kind="ExternalInput")
    wpT_d = nc.dram_tensor("wpT", [HL, C], BF16, kind="ExternalInput")
    out_d = nc.dram_tensor("out", [T, C], FP32, kind="ExternalOutput")

    with tile.TileContext(nc) as tc, ExitStack() as ctx:
        xt = ctx.enter_context(tc.tile_pool(name="xt", bufs=CT))
        wq = ctx.enter_context(tc.tile_pool(name="wq", bufs=CT))
        qk = ctx.enter_context(tc.tile_pool(name="qk", bufs=2 * NP))
        vv = ctx.enter_context(tc.tile_pool(name="vv", bufs=NCH))
        es = ctx.enter_context(tc.tile_pool(name="es", bufs=4))
        yt = ctx.enter_context(tc.tile_pool(name="yt", bufs=NP))
        wp = ctx.enter_context(tc.tile_pool(name="wp", bufs=NP))
        ob = ctx.enter_context(tc.tile_pool(name="ob", bufs=3))
        dn = ctx.enter_context(tc.tile_pool(name="dn", bufs=2))
        bc = ctx.enter_context(tc.tile_pool(name="bc", bufs=3))
        sc = ctx.enter_context(tc.tile_pool(name="sc", bufs=1))
        # PSUM budget (8 banks): ss = 2 x [128,1024] slabs (4 banks) shared
        # by qkv projections / scores / c_proj; py = 4 x [65,512] attV
        # accumulators (4 banks).
        ss = ctx.enter_context(tc.tile_pool(name="ss", bufs=2, space="PSUM"))
        py = ctx.enter_context(tc.tile_pool(name="py", bufs=4, space="PSUM"))

        # ---- weights + constants ----
        wqs = [wq.tile([128, 3 * HL], BF16, tag="wq", name="wtile")
               for _ in range(CT)]
        for blk in range(3):  # Q cols first so projections start early
            for c in range(CT):
                nc.gpsimd.dma_start(
                    out=wqs[c][:, blk * HL:(blk + 1) * HL],
                    in_=wqkvT_d[c * 128:(c + 1) * 128, blk * HL:(blk + 1) * HL])
        wps = []
        for p in range(NP):
            t_ = wp.tile([128, C], BF16, tag="wp", name="wptile")
            nc.gpsimd.dma_start(out=t_, in_=wpT_d[p * 128:(p + 1) * 128, :])
            wps.append(t_)
        bq_sb = sc.tile([128, NP], FP32, tag="bq")
        nc.sync.dma_start(out=bq_sb, in_=bq_d.ap().rearrange("(j p) -> p j", p=128))

        # lower-tri causal mask (1 where q >= k) and ones column source
        mask_sb = sc.tile([128, 128], BF16, tag="mask")
        nc.gpsimd.memset(mask_sb, 1.0)
        nc.gpsimd.affine_select(
            out=mask_sb, in_=mask_sb,
            compare_op=mybir.AluOpType.is_ge, fill=0.0, base=0,
            pattern=[[1, 128]], channel_multiplier=-1,
        )
        ones_sb = sc.tile([128, 4 * HPC], BF16, tag="ones")
        nc.gpsimd.memset(ones_sb, 1.0)

        # x streams in per 512-token chunk (vector+sync queues alternate)
        xts = [xt.tile([128, T], BF16, tag="xt", name="xtile")
               for _ in range(CT)]
        for tchunk in range(NCH):
            t0 = tchunk * 512
            for c in range(CT):
                nc.sync.dma_start(out=xts[c][:, t0:t0 + 512],
                                  in_=xT_d[c * 128:(c + 1) * 128, t0:t0 + 512])

        qk_tiles = [qk.tile([128, T], BF16, tag="qk", name="qktile")
                    for _ in range(2 * NP)]          # Q0, Q1, K0, K1
        yts = [yt.tile([128, T], BF16, tag="yt", name="ytile")
               for _ in range(NP)]
        vts = []
        for g in range(NCH):
            vt = vv.tile([128, 4, HPC, D + 1], BF16, tag="vv", name="vtile")
            nc.vector.tensor_copy(
                vt[:, :, :, D],
                ones_sb.rearrange("p (a h) -> p a h", a=4),
            )
            vts.append(vt)

        def emit_qkv(tchunk):
            t0 = tchunk * 512
            # Q for both pairs into one slab's halves, then K, then V
            for base, dst_i, bias in ((0, 0, True), (2 * HL // 2, 2, False)):
                slab = ss.tile([128, 1024], FP32, tag="ss", name="pqk")
                for p in range(NP):
                    col0 = base + p * 128
                    for c in range(CT):
                        nc.tensor.matmul(
                            slab[:, p * 512:p * 512 + 512],
                            wqs[c][:, col0:col0 + 128],
                            xts[c][:, t0:t0 + 512],
                            start=(c == 0), stop=(c == CT - 1),
                        )
                for p in range(NP):
                    dst = qk_tiles[dst_i + p][:, t0:t0 + 512]
                    src = slab[:, p * 512:p * 512 + 512]
                    if bias:
                        nc.vector.tensor_scalar_add(dst, src, bq_sb[:, p:p + 1])
                    else:
                        nc.vector.tensor_copy(dst, src)
            slab = ss.tile([128, 1024], FP32, tag="ss", name="pv")
            for tt4 in range(4):
                tt = 4 * tchunk + tt4
                for c in range(CT):
                    nc.tensor.matmul(
                        slab[:, tt4 * 256:tt4 * 256 + 256],
                        xts[c][:, tt * 128:(tt + 1) * 128],
                        wqs[c][:, 2 * HL:3 * HL],
                        start=(c == 0), stop=(c == CT - 1),
                    )
            nc.vector.tensor_copy(
                vts[tchunk][:, :, :, 0:D],
                slab.rearrange("p (a h d) -> p a h d", a=4, h=HPC),
            )

        def emit_attn(p, cg, den_t):
            """Attention for head pair p on q-chunk cg (q in [512cg, 512cg+512))."""
            q_t, k_t = qk_tiles[p], qk_tiles[2 + p]
            nkt = 4 * cg + 4
            accs = [py.tile([65, 512], FP32, tag="py", name="acc")
                    for _ in range(2)]
            for kt in range(nkt):
                qa = max(0, kt * 128 - cg * 512)   # local start within chunk
                w = 512 - qa
                q0 = cg * 512 + qa
                slab = ss.tile([128, 1024], FP32, tag="ss", name="pst")
                for h01 in range(2):
                    hb = 64 * h01
                    nc.tensor.matmul(
                        slab[:, h01 * 512:h01 * 512 + w],
                        k_t[hb:hb + 64, kt * 128:(kt + 1) * 128],
                        q_t[hb:hb + 64, q0:q0 + w],
                        start=True, stop=True,
                    )
                es_t = es.tile([128, 2, 512], BF16, tag="es", name="estile")
                nc.scalar.activation(
                    es_t[:, :, 0:w],
                    slab.rearrange("x (h q) -> x h q", h=2)[:, :, 0:w],
                    AF.Exp, scale=0.125,
                )
                if kt >= 4 * cg:  # diagonal block: zero k > q (gpsimd, SBUF)
                    for h01 in range(2):
                        nc.gpsimd.tensor_mul(
                            es_t[:, h01, 0:128], es_t[:, h01, 0:128], mask_sb)
                for h01 in range(2):
                    nc.tensor.matmul(
                        accs[h01][:, qa:512],
                        vts[kt // 4][:, kt % 4, 2 * p + h01, :],
                        es_t[:, h01, 0:w],
                        start=(kt == 0), stop=(kt == nkt - 1),
                    )
            cs = slice(cg * 512, cg * 512 + 512)
            for h01 in range(2):
                r = 32 * (2 * p + h01)
                nc.vector.tensor_copy(
                    yts[p][64 * h01:64 * h01 + 64, cs], accs[h01][0:64, :])
                nc.vector.tensor_copy(
                    den_t[r:r + 1, :], accs[h01][64:65, :])

        def emit_norm_cproj(cg, den_t):
            cs = slice(cg * 512, cg * 512 + 512)
            nc.vector.reciprocal_approx_fast(den_t, den_t)
            for p in range(NP):
                for h01 in range(2):
                    r = 32 * (2 * p + h01)
                    rr = bc.tile([1, 512], FP32, tag="rr", name="rrow")
                    nc.sync.dma_start(out=rr, in_=den_t[r:r + 1, :])
                    bc_t = bc.tile([128, 512], FP32, tag="bc", name="bct")
                    nc.gpsimd.partition_broadcast(bc_t, rr)
                    dst = yts[p][64 * h01:64 * h01 + 64, cs]
                    nc.gpsimd.tensor_mul(dst, dst, bc_t[64 * h01:64 * h01 + 64, :])
            for tt in range(4 * cg, 4 * cg + 4):
                po = ss.tile([128, 1024], FP32, tag="ss", name="po")
                for s01 in range(2):
                    for p in range(NP):
                        nc.tensor.matmul(
                            po[:, s01 * 512:(s01 + 1) * 512],
                            yts[p][:, tt * 128:(tt + 1) * 128],
                            wps[p][:, s01 * 512:(s01 + 1) * 512],
                            start=(p == 0), stop=(p == NP - 1),
                        )
                ot = ob.tile([128, C], FP32, tag="ob", name="otile")
                nc.vector.tensor_copy(ot, po)
                nc.sync.dma_start(out=out_d[tt * 128:(tt + 1) * 128, :], in_=ot)

        # ---- pipelined emission ----
        emit_qkv(0)
        emit_qkv(1)
        for cg in range(NCH):
            den_t = dn.tile([128, 512], FP32, tag="dn", name="dent")
            emit_attn(0, cg, den_t)
            emit_attn(1, cg, den_t)
            if cg + 2 < NCH:
                emit_qkv(cg + 2)
            emit_norm_cproj(cg, den_t)

    nc.compile()
    return nc


_NC_CACHE = {}


def _get_nc(T=T_FULL):
    if T not in _NC_CACHE:
        _NC_CACHE[T] = build_bass(T)
    return _NC_CACHE[T]


def make_in_maps(x, w_attn, b_attn, w_proj, T=T_FULL):
    bf = ml_dtypes.bfloat16
    x = np.ascontiguousarray(np.asarray(x, np.float32))
    w_attn = np.asarray(w_attn, np.float32)
    b_attn = np.asarray(b_attn, np.float32)
    w_proj = np.asarray(w_proj, np.float32)
    xTs = [np.ascontiguousarray(x[b].T.astype(bf)) for b in range(x.shape[0])]
    in_maps = []
    for core in range(NCORES):
        b, j = core // CPG, core % CPG
        r0 = j * HL
        wq_s = w_attn[r0:r0 + HL]
        wk_s = w_attn[C + r0:C + r0 + HL]
        wv_s = w_attn[2 * C + r0:2 * C + r0 + HL]
        in_maps.append({
            "xT": xTs[b],
            "wqkvT": np.ascontiguousarray(
                np.concatenate([wq_s, wk_s, wv_s], axis=0).T.astype(bf)),
            "bq": np.ascontiguousarray(b_attn[r0:r0 + HL]),
            "wpT": np.ascontiguousarray(w_proj[:, r0:r0 + HL].T.astype(bf)),
        })
    return in_maps


def run_device(x, w_attn, b_attn, w_proj, b_proj, T=T_FULL, **spmd_kwargs):
    nc = _get_nc(T)
    in_maps = make_in_maps(x, w_attn, b_attn, w_proj, T)
    res = run_bass_kernel_spmd(nc, in_maps, core_ids=list(range(NCORES)),
                               **spmd_kwargs)
    outs = [r["out"] for r in res.results]
    b_eff = (np.asarray(b_proj, np.float32)
             + np.asarray(w_proj, np.float32) @ np.asarray(b_attn, np.float32)[2 * C:])
    full = np.stack(
        [sum(outs[b * CPG:(b + 1) * CPG][1:], outs[b * CPG]) + b_eff
         for b in range(B)]
    ).astype(np.float32)
    return full, res


def kernel(x, w_attn, b_attn, w_proj, b_proj):
    out, _ = run_device(x, w_attn, b_attn, w_proj, b_proj)
    return out


# revision 6
# speedup vs baseline: 1.9106x; 1.9106x over previous
"""Causal self-attention (dense transformer block) on 8 Trainium2 NeuronCores.

Sharding: tensor-parallel over heads x data-parallel over batch.
  - 8 cores = 2 batch groups x 4 cores; each core owns 1 batch element and
    4 of the 16 heads (head_dim 64 -> 256 local channels, 2 "pairs" of heads).
  - Host pre-transposes x and the weight slices; everything ships as bf16
    (verified 3.5e-3 rel err vs the 2e-2 gate) so input DMA is ~6.3 MB/core.
  - Each core computes qkv, causal attention in "S^T" layout (scores[k, q],
    k on partitions), and its partial c_proj; host sums 4 partials per batch.

Math notes (carried over from the fp32r version):
  - k-bias cancels in softmax; v-bias passes through to a constant output
    offset w_proj @ b_v added on host together with b_proj.
  - Softmax skips max-subtraction: |scores/8| <~ 3 for this distribution.
  - V carries a ones column so softmax denominators fall out of the attV
    matmul (row 64 of the PSUM accumulator).

Schedule notes (what changed vs the 279us version):
  - x streams in by 512-token chunks and qkv projections start per chunk,
    so the PE is busy ~7us in instead of ~40us.
  - Scores matmuls for the two heads of a pair contract over 64 partitions
    each and are emitted back-to-back at PE row-tiles 0/64 (tile_position
    auto-derives from base partitions), so they run concurrently.
  - exp for both heads of a pair is ONE ScalarE activation over a strided
    [128, 2, w] view of the scores PSUM slab -> halves ACT instruction
    overhead (352 cyc/op). es is bf16.
  - Causal diag masking = 2 cheap DVE multiplies with a precomputed
    lower-tri bf16 mask (off GpSimd's affine_select path).
  - Attention runs chunk-major (q-chunks of 512), pairs alternating, with
    qkv projection chunks and per-chunk c_proj interleaved in emission
    order so the PE always has dense independent matmul work -> the HAM
    clock gate stays at 8/8 (baseline spent 136us at 4/8).
  - Softmax denominators: one [4, 512] reciprocal per chunk (DVE recip
    cost is free-dim-bound, so batching partitions is ~free), gpsimd
    partition_broadcast, in-place bf16 normalize of y.
  - c_proj per chunk as soon as both pairs' y are normalized; output
    streams out per 128-token tile.
"""

import numpy as np
from contextlib import ExitStack

import ml_dtypes

import concourse.bass as bass
import concourse.tile as tile
from concourse import bacc, mybir
from concourse.bass_utils import run_bass_kernel_spmd

FP32 = mybir.dt.float32
BF16 = mybir.dt.bfloat16
AF = mybir.ActivationFunctionType

B, T_FULL, C = 2, 2048, 1024
H, D = 16, 64
NCORES = 8
CPG = 4          # cores per batch group
HPC = H // CPG   # heads per core = 4
HL = HPC * D     # local channels = 256
NP = 2           # head pairs per core
CT = C // 128    # contraction tiles = 8


def build_bass(T=T_FULL):
    """Emit the SPMD Bass/Tile program for one core."""
    assert T % 512 == 0
    TT = T // 128          # 128-token t-tiles (16)
    NCH = T // 512         # 512-token chunks (4)

    nc = bacc.Bacc("TRN2", target_bir_lowering=False, debug=False,
                   num_devices=NCORES)

    xT_d = nc.dram_tensor("xT", [C, T], BF16, kind="ExternalInput")
    wqkvT_d = nc.dram_tensor("wqkvT", [C, 3 * HL], BF16, kind="ExternalInput")
    bq_d = nc.dram_tensor("bq", [HL], FP32, kind="ExternalInput")
    wpT_d = nc.dram_tensor("wpT", [HL, C], BF16, kind="ExternalInput")
    out_d = nc.dram_tensor("out", [T, C], FP32, kind="ExternalOutput")

    with tile.TileContext(nc) as tc, ExitStack() as ctx:
        xt = ctx.enter_context(tc.tile_pool(name="xt", bufs=CT))
        wq = ctx.enter_context(tc.tile_pool(name="wq", bufs=CT))
        qk = ctx.enter_context(tc.tile_pool(name="qk", bufs=2 * NP))
        vv = ctx.enter_context(tc.tile_pool(name="vv", bufs=NCH))
        es = ctx.enter_context(tc.tile_pool(name="es", bufs=4))
        yt = ctx.enter_context(tc.tile_pool(name="yt", bufs=NP))
        wp = ctx.enter_context(tc.tile_pool(name="wp", bufs=NP))
        ob = ctx.enter_context(tc.tile_pool(name="ob", bufs=3))
        dn = ctx.enter_context(tc.tile_pool(name="dn", bufs=2))
        bc = ctx.enter_context(tc.tile_pool(name="bc", bufs=3))
        sc = ctx.enter_context(tc.tile_pool(name="sc", bufs=1))
        # PSUM budget (8 banks): ss = 2 x [128,1024] slabs (4 banks) shared
        # by qkv projections / scores / c_proj; py = 4 x [65,512] attV
        # accumulators (4 banks).
        ss = ctx.enter_context(tc.tile_pool(name="ss", bufs=2, space="PSUM"))
        py = ctx.enter_context(tc.tile_pool(name="py", bufs=4, space="PSUM"))

        # ---- weights + constants ----
        wqs = [wq.tile([128, 3 * HL], BF16, tag="wq", name="wtile")
               for _ in range(CT)]
        for blk in range(3):  # Q cols first so projections start early
            for c in range(CT):
                nc.gpsimd.dma_start(
                    out=wqs[c][:, blk * HL:(blk + 1) * HL],
                    in_=wqkvT_d[c * 128:(c + 1) * 128, blk * HL:(blk + 1) * HL])
        wps = []
        for p in range(NP):
            t_ = wp.tile([128, C], BF16, tag="wp", name="wptile")
            nc.gpsimd.dma_start(out=t_, in_=wpT_d[p * 128:(p + 1) * 128, :])
            wps.append(t_)
        bq_sb = sc.tile([128, NP], FP32, tag="bq")
        nc.sync.dma_start(out=bq_sb, in_=bq_d.ap().rearrange("(j p) -> p j", p=128))

        ones_sb = sc.tile([128, 4 * HPC], BF16, tag="ones")
        nc.gpsimd.memset(ones_sb, 1.0)

        # x streams in per 512-token chunk (vector+sync queues alternate)
        xts = [xt.tile([128, T], BF16, tag="xt", name="xtile")
               for _ in range(CT)]
        for tchunk in range(NCH):
            t0 = tchunk * 512
            for c in range(CT):
                nc.sync.dma_start(out=xts[c][:, t0:t0 + 512],
                                  in_=xT_d[c * 128:(c + 1) * 128, t0:t0 + 512])

        qk_tiles = [qk.tile([128, T], BF16, tag="qk", name="qktile")
                    for _ in range(2 * NP)]          # Q0, Q1, K0, K1
        yts = [yt.tile([128, T], BF16, tag="yt", name="ytile")
               for _ in range(NP)]
        vts = []
        for g in range(NCH):
            vt = vv.tile([128, 4, HPC, D + 1], BF16, tag="vv", name="vtile")
            nc.vector.tensor_copy(
                vt[:, :, :, D],
                ones_sb.rearrange("p (a h) -> p a h", a=4),
            )
            vts.append(vt)

        def emit_qkv(tchunk):
            t0 = tchunk * 512
            # Q for both pairs into one slab's halves, then K, then V
            for base, dst_i, bias in ((0, 0, True), (2 * HL // 2, 2, False)):
                slab = ss.tile([128, 1024], FP32, tag="ss", name="pqk")
                for p in range(NP):
                    col0 = base + p * 128
                    for c in range(CT):
                        nc.tensor.matmul(
                            slab[:, p * 512:p * 512 + 512],
                            wqs[c][:, col0:col0 + 128],
                            xts[c][:, t0:t0 + 512],
                            start=(c == 0), stop=(c == CT - 1),
                        )
                for p in range(NP):
                    dst = qk_tiles[dst_i + p][:, t0:t0 + 512]
                    src = slab[:, p * 512:p * 512 + 512]
                    if bias:
                        nc.vector.tensor_scalar_add(dst, src, bq_sb[:, p:p + 1])
                    else:
                        nc.vector.tensor_copy(dst, src)
            slab = ss.tile([128, 1024], FP32, tag="ss", name="pv")
            for tt4 in range(4):
                tt = 4 * tchunk + tt4
                for c in range(CT):
                    nc.tensor.matmul(
                        slab[:, tt4 * 256:tt4 * 256 + 256],
                        xts[c][:, tt * 128:(tt + 1) * 128],
                        wqs[c][:, 2 * HL:3 * HL],
                        start=(c == 0), stop=(c == CT - 1),
                    )
            nc.vector.tensor_copy(
                vts[tchunk][:, :, :, 0:D],
                slab.rearrange("p (a h d) -> p a h d", a=4, h=HPC),
            )

        def emit_attn(p, cg, den_t):
            """Attention for head pair p on q-chunk cg (q in [512cg, 512cg+512))."""
            q_t, k_t = qk_tiles[p], qk_tiles[2 + p]
            nkt = 4 * cg + 4
            accs = [py.tile([65, 512], FP32, tag="py", name="acc")
                    for _ in range(2)]
            for kt in range(nkt):
                qa = max(0, kt * 128 - cg * 512)   # local start within chunk
                w = 512 - qa
                q0 = cg * 512 + qa
                slab = ss.tile([128, 1024], FP32, tag="ss", name="pst")
                for h01 in range(2):
                    hb = 64 * h01
                    nc.tensor.matmul(
                        slab[:, h01 * 512:h01 * 512 + w],
                        k_t[hb:hb + 64, kt * 128:(kt + 1) * 128],
                        q_t[hb:hb + 64, q0:q0 + w],
                        start=True, stop=True,
                    )
                es_t = es.tile([128, 2, 512], BF16, tag="es", name="estile")
                nc.scalar.activation(
                    es_t[:, :, 0:w],
                    slab.rearrange("x (h q) -> x h q", h=2)[:, :, 0:w],
                    AF.Exp, scale=0.125,
                )
                if kt >= 4 * cg:  # diagonal block: zero k > q (gpsimd, SBUF)
                    for h01 in range(2):
                        nc.gpsimd.affine_select(
                            out=es_t[:, h01, 0:128], in_=es_t[:, h01, 0:128],
                            compare_op=mybir.AluOpType.is_ge,
                            fill=0.0, base=0,
                            pattern=[[1, 128]], channel_multiplier=-1,
                        )
                for h01 in range(2):
                    nc.tensor.matmul(
                        accs[h01][:, qa:512],
                        vts[kt // 4][:, kt % 4, 2 * p + h01, :],
                        es_t[:, h01, 0:w],
                        start=(kt == 0), stop=(kt == nkt - 1),
                    )
            cs = slice(cg * 512, cg * 512 + 512)
            for h01 in range(2):
                r = 32 * (2 * p + h01)
                nc.vector.tensor_copy(
                    yts[p][64 * h01:64 * h01 + 64, cs], accs[h01][0:64, :])
                nc.vector.tensor_copy(
                    den_t[r:r + 1, :], accs[h01][64:65, :])

        def emit_norm_cproj(cg, den_t, den_b):
            cs = slice(cg * 512, cg * 512 + 512)
            nc.vector.reciprocal_approx_fast(den_t, den_t)
            nc.vector.tensor_copy(den_b, den_t)  # fp32 -> bf16
            for p in range(NP):
                for h01 in range(2):
                    r = 32 * (2 * p + h01)
                    rr = bc.tile([1, 512], BF16, tag="rr", name="rrow")
                    nc.sync.dma_start(out=rr, in_=den_b[r:r + 1, :])
                    bc_t = bc.tile([128, 512], BF16, tag="bc", name="bct")
                    nc.gpsimd.partition_broadcast(bc_t, rr)
                    dst = yts[p][64 * h01:64 * h01 + 64, cs]
                    nc.vector.tensor_mul(dst, dst, bc_t[64 * h01:64 * h01 + 64, :])
            for tt in range(4 * cg, 4 * cg + 4):
                po = ss.tile([128, 1024], FP32, tag="ss", name="po")
                for s01 in range(2):
                    for p in range(NP):
                        nc.tensor.matmul(
                            po[:, s01 * 512:(s01 + 1) * 512],
                            yts[p][:, tt * 128:(tt + 1) * 128],
                            wps[p][:, s01 * 512:(s01 + 1) * 512],
                            start=(p == 0), stop=(p == NP - 1),
                        )
                ot = ob.tile([128, C], FP32, tag="ob", name="otile")
                nc.vector.tensor_copy(ot, po)
                nc.sync.dma_start(out=out_d[tt * 128:(tt + 1) * 128, :], in_=ot)

        # ---- pipelined emission ----
        emit_qkv(0)
        emit_qkv(1)
        for cg in range(NCH):
            den_t = dn.tile([128, 512], FP32, tag="dn", name="dent")
            den_b = dn.tile([128, 512], BF16, tag="dnb", name="denb")
            emit_attn(0, cg, den_t)
            emit_attn(1, cg, den_t)
            if cg + 2 < NCH:
                emit_qkv(cg + 2)
            emit_norm_cproj(cg, den_t, den_b)

    nc.compile()
    return nc


_NC_CACHE = {}


def _get_nc(T=T_FULL):
    if T not in _NC_CACHE:
        _NC_CACHE[T] = build_bass(T)
    return _NC_CACHE[T]


def make_in_maps(x, w_attn, b_attn, w_proj, T=T_FULL):
    bf = ml_dtypes.bfloat16
    x = np.ascontiguousarray(np.asarray(x, np.float32))
    w_attn = np.asarray(w_attn, np.float32)
    b_attn = np.asarray(b_attn, np.float32)
    w_proj = np.asarray(w_proj, np.float32)
    xTs = [np.ascontiguousarray(x[b].T.astype(bf)) for b in range(x.shape[0])]
    in_maps = []
    for core in range(NCORES):
        b, j = core // CPG, core % CPG
        r0 = j * HL
        wq_s = w_attn[r0:r0 + HL]
        wk_s = w_attn[C + r0:C + r0 + HL]
        wv_s = w_attn[2 * C + r0:2 * C + r0 + HL]
        in_maps.append({
            "xT": xTs[b],
            "wqkvT": np.ascontiguousarray(
                np.concatenate([wq_s, wk_s, wv_s], axis=0).T.astype(bf)),
            "bq": np.ascontiguousarray(b_attn[r0:r0 + HL]),
            "wpT": np.ascontiguousarray(w_proj[:, r0:r0 + HL].T.astype(bf)),
        })
    return in_maps


def run_device(x, w_attn, b_attn, w_proj, b_proj, T=T_FULL, **spmd_kwargs):
    nc = _get_nc(T)
    in_maps = make_in_maps(x, w_attn, b_attn, w_proj, T)
    res = run_bass_kernel_spmd(nc, in_maps, core_ids=list(range(NCORES)),
                               **spmd_kwargs)
    outs = [r["out"] for r in res.results]
    b_eff = (np.asarray(b_proj, np.float32)
             + np.asarray(w_proj, np.float32) @ np.asarray(b_attn, np.float32)[2 * C:])
    full = np.stack(
        [sum(outs[b * CPG:(b + 1) * CPG][1:], outs[b * CPG]) + b_eff
         for b in range(B)]
    ).astype(np.float32)
    return full, res


def kernel(x, w_attn, b_attn, w_proj, b_proj):
    out, _ = run_device(x, w_attn, b_attn, w_proj, b_proj)
    return out


# revision 10
# speedup vs baseline: 2.0780x; 1.0876x over previous
"""Causal self-attention (dense transformer block) on 8 Trainium2 NeuronCores.

Sharding: tensor-parallel over heads x data-parallel over batch.
  - 8 cores = 2 batch groups x 4 cores; each core owns 1 batch element and
    4 of the 16 heads (head_dim 64 -> 256 local channels, 2 "pairs" of heads).
  - Host pre-transposes x and the weight slices; everything ships as bf16
    (3.8e-3 rel err vs the 2e-2 gate) so input DMA is ~6.3 MB/core.
  - Each core computes qkv, causal attention in "S^T" layout (scores[k, q],
    k on partitions), and its partial c_proj; host sums 4 partials per batch.

Math notes:
  - k-bias cancels in softmax; v-bias passes through to a constant output
    offset w_proj @ b_v added on host together with b_proj.
  - Softmax skips max-subtraction: |scores/8| <~ 3 for this distribution.
  - V carries a ones column so softmax denominators fall out of the attV
    matmul (row 64 of the PSUM accumulator).

Schedule (the whole game is keeping PE dense and the HAM clock warm while
ScalarE streams exp):
  - PE warm-up matmuls on a memset tile run while the first DMAs land.
  - x streams per 512-token chunk on two DMA queues (sync + scalar);
    Q/K/V weight column-blocks load Q-first on the gpsimd queue.
  - Scores for the two heads of a pair run CONCURRENTLY as PE row-tiles
    0/64 (tile_position auto-derives from base partitions).
  - exp for both heads of a pair is ONE ScalarE activation over a strided
    [128, 2, w] view of the scores PSUM slab; es is bf16; causal diag
    masking via gpsimd affine_select (off the PE/ACT/DVE critical path).
  - Attention is chunk-major with pairs back-to-back; each pair's softmax
    denominators are reciprocal'd (DVE approx, direct from PSUM row 64),
    partition-broadcast and multiplied into y WHILE the other pair's
    attention runs, so c_proj starts right after the second pair finishes.
  - Deferred K/V projections and split c_proj t-tiles are emitted INSIDE
    the late attention kt-loops as PE filler, since exp (ScalarE) is the
    per-kt critical stage there.
  - Output partials are bf16 (host sums in fp32): halves the output DMA
    and the PSUM->SBUF copies.
"""

import numpy as np
from contextlib import ExitStack

import ml_dtypes

import concourse.bass as bass
import concourse.tile as tile
from concourse import bacc, mybir
from concourse.bass_utils import run_bass_kernel_spmd

FP32 = mybir.dt.float32
BF16 = mybir.dt.bfloat16
AF = mybir.ActivationFunctionType

B, T_FULL, C = 2, 2048, 1024
H, D = 16, 64
NCORES = 8
CPG = 4          # cores per batch group
HPC = H // CPG   # heads per core = 4
HL = HPC * D     # local channels = 256
NP = 2           # head pairs per core
CT = C // 128    # contraction tiles = 8


def build_bass(T=T_FULL):
    """Emit the SPMD Bass/Tile program for one core."""
    assert T % 512 == 0
    NCH = T // 512         # 512-token chunks (4)

    nc = bacc.Bacc("TRN2", target_bir_lowering=False, debug=False,
                   num_devices=NCORES)

    xT_d = nc.dram_tensor("xT", [C, T], BF16, kind="ExternalInput")
    wqkvT_d = nc.dram_tensor("wqkvT", [C, 3 * HL], BF16, kind="ExternalInput")
    bq_d = nc.dram_tensor("bq", [HL], FP32, kind="ExternalInput")
    wpT_d = nc.dram_tensor("wpT", [HL, C], BF16, kind="ExternalInput")
    out_d = nc.dram_tensor("out", [T, C], BF16, kind="ExternalOutput")

    with tile.TileContext(nc) as tc, ExitStack() as ctx:
        xt = ctx.enter_context(tc.tile_pool(name="xt", bufs=CT))
        wq = ctx.enter_context(tc.tile_pool(name="wq", bufs=CT))
        qk = ctx.enter_context(tc.tile_pool(name="qk", bufs=2 * NP))
        vv = ctx.enter_context(tc.tile_pool(name="vv", bufs=NCH))
        es = ctx.enter_context(tc.tile_pool(name="es", bufs=4))
        yt = ctx.enter_context(tc.tile_pool(name="yt", bufs=NP))
        wp = ctx.enter_context(tc.tile_pool(name="wp", bufs=NP))
        ob = ctx.enter_context(tc.tile_pool(name="ob", bufs=3))
        dn = ctx.enter_context(tc.tile_pool(name="dn", bufs=2))
        bc = ctx.enter_context(tc.tile_pool(name="bc", bufs=3))
        sc = ctx.enter_context(tc.tile_pool(name="sc", bufs=1))
        # PSUM budget (8 banks): ss = 2 x [128,1024] slabs (4 banks) for
        # qkv projections + scores; py = 3 x [65,512] attV accumulators;
        # pp = 1 x [128,512] c_proj slab (own pool so next chunk's scores
        # never wait on the c_proj->normalize chain through the ss ring).
        ss = ctx.enter_context(tc.tile_pool(name="ss", bufs=2, space="PSUM"))
        py = ctx.enter_context(tc.tile_pool(name="py", bufs=3, space="PSUM"))
        pp = ctx.enter_context(tc.tile_pool(name="pp", bufs=1, space="PSUM"))

        # ---- PE warm-up: spin the HAM clock while DMAs land ----
        warm_sb = sc.tile([128, 512], BF16, tag="warm")
        nc.gpsimd.memset(warm_sb, 0.0)
        wslab = ss.tile([128, 1024], FP32, tag="ss", name="wslab")
        for i in range(10):
            nc.tensor.matmul(wslab[:, 0:512], warm_sb[:, 0:128], warm_sb,
                             start=(i == 0), stop=(i == 9))

        # ---- weights + constants ----
        wqs = [wq.tile([128, 3 * HL], BF16, tag="wq", name="wtile")
               for _ in range(CT)]
        for blk in range(3):  # Q cols first so projections start early
            for c in range(CT):
                nc.gpsimd.dma_start(
                    out=wqs[c][:, blk * HL:(blk + 1) * HL],
                    in_=wqkvT_d[c * 128:(c + 1) * 128, blk * HL:(blk + 1) * HL])
        wps = []
        for p in range(NP):
            t_ = wp.tile([128, C], BF16, tag="wp", name="wptile")
            nc.gpsimd.dma_start(out=t_, in_=wpT_d[p * 128:(p + 1) * 128, :])
            wps.append(t_)
        bq_sb = sc.tile([128, NP], FP32, tag="bq")
        nc.sync.dma_start(out=bq_sb, in_=bq_d.ap().rearrange("(j p) -> p j", p=128))

        ones_sb = sc.tile([128, 4 * HPC], BF16, tag="ones")
        nc.gpsimd.memset(ones_sb, 1.0)

        # x streams in per 512-token chunk, split across sync+scalar queues
        xts = [xt.tile([128, T], BF16, tag="xt", name="xtile")
               for _ in range(CT)]
        for tchunk in range(NCH):
            t0 = tchunk * 512
            for c in range(CT):
                eng = nc.sync if (c % 2 == 0 or tchunk >= 2) else nc.scalar
                eng.dma_start(out=xts[c][:, t0:t0 + 512],
                              in_=xT_d[c * 128:(c + 1) * 128, t0:t0 + 512])

        qk_tiles = [qk.tile([128, T], BF16, tag="qk", name="qktile")
                    for _ in range(2 * NP)]          # Q0, Q1, K0, K1
        yts = [yt.tile([128, T], BF16, tag="yt", name="ytile")
               for _ in range(NP)]
        vts = []
        for g in range(NCH):
            vt = vv.tile([128, 4, HPC, D + 1], BF16, tag="vv", name="vtile")
            nc.vector.tensor_copy(
                vt[:, :, :, D],
                ones_sb.rearrange("p (a h) -> p a h", a=4),
            )
            vts.append(vt)

        def emit_q(tchunk):
            t0 = tchunk * 512
            slab = ss.tile([128, 1024], FP32, tag="ss", name="pq")
            for p in range(NP):
                for c in range(CT):
                    nc.tensor.matmul(
                        slab[:, p * 512:p * 512 + 512],
                        wqs[c][:, p * 128:p * 128 + 128],
                        xts[c][:, t0:t0 + 512],
                        start=(c == 0), stop=(c == CT - 1),
                    )
            for p in range(NP):
                nc.vector.tensor_scalar_add(
                    qk_tiles[p][:, t0:t0 + 512],
                    slab[:, p * 512:p * 512 + 512], bq_sb[:, p:p + 1])

        def emit_kv(tchunk):
            t0 = tchunk * 512
            slab = ss.tile([128, 1024], FP32, tag="ss", name="pk")
            for p in range(NP):
                for c in range(CT):
                    nc.tensor.matmul(
                        slab[:, p * 512:p * 512 + 512],
                        wqs[c][:, HL + p * 128:HL + p * 128 + 128],
                        xts[c][:, t0:t0 + 512],
                        start=(c == 0), stop=(c == CT - 1),
                    )
            for p in range(NP):
                nc.vector.tensor_copy(
                    qk_tiles[2 + p][:, t0:t0 + 512],
                    slab[:, p * 512:p * 512 + 512])
            slab = ss.tile([128, 1024], FP32, tag="ss", name="pv")
            for tt4 in range(4):
                tt = 4 * tchunk + tt4
                for c in range(CT):
                    nc.tensor.matmul(
                        slab[:, tt4 * 256:tt4 * 256 + 256],
                        xts[c][:, tt * 128:(tt + 1) * 128],
                        wqs[c][:, 2 * HL:3 * HL],
                        start=(c == 0), stop=(c == CT - 1),
                    )
            nc.vector.tensor_copy(
                vts[tchunk][:, :, :, 0:D],
                slab.rearrange("p (a h d) -> p a h d", a=4, h=HPC),
            )

        def emit_attn(p, cg, den_t, fillers=()):
            """Attention for head pair p on q-chunk cg.

            fillers: list of (after_kt, fn) emission hooks for PE filler work.
            Writes unnormalized y into yts and 1/denominator into den_t
            (partitions 0 / 32 for h0 / h1).
            """
            q_t, k_t = qk_tiles[p], qk_tiles[2 + p]
            nkt = 4 * cg + 4
            accs = [py.tile([65, 512], FP32, tag="py", name="acc")
                    for _ in range(2)]
            fillers = list(fillers)
            for kt in range(nkt):
                while fillers and fillers[0][0] <= kt:
                    fillers.pop(0)[1]()
                qa = max(0, kt * 128 - cg * 512)   # local start within chunk
                w = 512 - qa
                q0 = cg * 512 + qa
                slab = ss.tile([128, 1024], FP32, tag="ss", name="pst")
                for h01 in range(2):
                    hb = 64 * h01
                    nc.tensor.matmul(
                        slab[:, h01 * 512:h01 * 512 + w],
                        k_t[hb:hb + 64, kt * 128:(kt + 1) * 128],
                        q_t[hb:hb + 64, q0:q0 + w],
                        start=True, stop=True,
                    )
                es_t = es.tile([128, 2, 512], BF16, tag="es", name="estile")
                nc.scalar.activation(
                    es_t[:, :, 0:w],
                    slab.rearrange("x (h q) -> x h q", h=2)[:, :, 0:w],
                    AF.Exp, scale=0.125,
                )
                if kt >= 4 * cg:  # diagonal block: zero k > q (gpsimd, SBUF)
                    for h01 in range(2):
                        nc.gpsimd.affine_select(
                            out=es_t[:, h01, 0:128], in_=es_t[:, h01, 0:128],
                            compare_op=mybir.AluOpType.is_ge,
                            fill=0.0, base=0,
                            pattern=[[1, 128]], channel_multiplier=-1,
                        )
                for h01 in range(2):
                    nc.tensor.matmul(
                        accs[h01][:, qa:512],
                        vts[kt // 4][:, kt % 4, 2 * p + h01, :],
                        es_t[:, h01, 0:w],
                        start=(kt == 0), stop=(kt == nkt - 1),
                    )
            for f in fillers:
                f[1]()
            cs = slice(cg * 512, cg * 512 + 512)
            for h01 in range(2):
                nc.vector.tensor_copy(
                    yts[p][64 * h01:64 * h01 + 64, cs], accs[h01][0:64, :])
                nc.vector.tensor_copy(
                    den_t[32 * h01:32 * h01 + 1, :], accs[h01][64:65, :])
            nc.vector.reciprocal_approx_fast(den_t[0:33, :], den_t[0:33, :])

        def emit_norm(p, cg, den_t, den_b):
            """Broadcast 1/den and scale this pair's y in place (bf16)."""
            cs = slice(cg * 512, cg * 512 + 512)
            nc.vector.tensor_copy(den_b, den_t[0:33, :])  # fp32 -> bf16
            for h01 in range(2):
                rr = bc.tile([1, 512], BF16, tag="rr", name="rrow")
                nc.sync.dma_start(out=rr, in_=den_b[32 * h01:32 * h01 + 1, :])
                bc_t = bc.tile([128, 512], BF16, tag="bc", name="bct")
                nc.gpsimd.partition_broadcast(bc_t, rr)
                dst = yts[p][64 * h01:64 * h01 + 64, cs]
                nc.vector.tensor_mul(dst, dst, bc_t[64 * h01:64 * h01 + 64, :])

        def emit_cproj(tts):
            for tt in tts:
                ot = ob.tile([128, C], BF16, tag="ob", name="otile")
                for s01 in range(2):
                    po = pp.tile([128, 512], FP32, tag="pp", name="po")
                    for p in range(NP):
                        nc.tensor.matmul(
                            po,
                            yts[p][:, tt * 128:(tt + 1) * 128],
                            wps[p][:, s01 * 512:(s01 + 1) * 512],
                            start=(p == 0), stop=(p == NP - 1),
                        )
                    nc.vector.tensor_copy(ot[:, s01 * 512:(s01 + 1) * 512], po)
                nc.sync.dma_start(out=out_d[tt * 128:(tt + 1) * 128, :], in_=ot)

        # ---- pipelined emission ----
        def den_tiles():
            t_ = dn.tile([128, 512], FP32, tag="dn", name="dent")
            b_ = dn.tile([33, 512], BF16, tag="dnb", name="denb")
            return t_, b_

        emit_q(0); emit_kv(0); emit_q(1); emit_kv(1)
        for cg in range(NCH):
            dts = [den_tiles() for _ in range(NP)]
            if cg == 0:
                emit_attn(0, cg, dts[0][0])
                emit_norm(0, cg, *dts[0])
                emit_attn(1, cg, dts[1][0])
                emit_norm(1, cg, *dts[1])
                emit_q(2); emit_kv(2)
                emit_cproj(range(0, 4))
            elif cg == 1:
                emit_attn(0, cg, dts[0][0])
                emit_norm(0, cg, *dts[0])
                emit_attn(1, cg, dts[1][0], fillers=[(4, lambda: emit_q(3))])
                emit_norm(1, cg, *dts[1])
                emit_cproj(range(4, 8))
            elif cg == 2:
                emit_attn(0, cg, dts[0][0], fillers=[(4, lambda: emit_kv(3))])
                emit_norm(0, cg, *dts[0])
                emit_attn(1, cg, dts[1][0])
                emit_norm(1, cg, *dts[1])
                emit_cproj(range(8, 10))
            else:
                emit_attn(0, cg, dts[0][0],
                          fillers=[(4, lambda: emit_cproj(range(10, 11))),
                                   (10, lambda: emit_cproj(range(11, 12)))])
                emit_norm(0, cg, *dts[0])
                emit_attn(1, cg, dts[1][0])
                emit_norm(1, cg, *dts[1])
                emit_cproj(range(12, 16))

    nc.compile()
    return nc


_NC_CACHE = {}


def _get_nc(T=T_FULL):
    if T not in _NC_CACHE:
        _NC_CACHE[T] = build_bass(T)
    return _NC_CACHE[T]


def make_in_maps(x, w_attn, b_attn, w_proj, T=T_FULL):
    bf = ml_dtypes.bfloat16
    x = np.ascontiguousarray(np.asarray(x, np.float32))
    w_attn = np.asarray(w_attn, np.float32)
    b_attn = np.asarray(b_attn, np.float32)
    w_proj = np.asarray(w_proj, np.float32)
    xTs = [np.ascontiguousarray(x[b].T.astype(bf)) for b in range(x.shape[0])]
    in_maps = []
    for core in range(NCORES):
        b, j = core // CPG, core % CPG
        r0 = j * HL
        wq_s = w_attn[r0:r0 + HL]
        wk_s = w_attn[C + r0:C + r0 + HL]
        wv_s = w_attn[2 * C + r0:2 * C + r0 + HL]
        in_maps.append({
            "xT": xTs[b],
            "wqkvT": np.ascontiguousarray(
                np.concatenate([wq_s, wk_s, wv_s], axis=0).T.astype(bf)),
            "bq": np.ascontiguousarray(b_attn[r0:r0 + HL]),
            "wpT": np.ascontiguousarray(w_proj[:, r0:r0 + HL].T.astype(bf)),
        })
    return in_maps


def run_device(x, w_attn, b_attn, w_proj, b_proj, T=T_FULL, **spmd_kwargs):
    nc = _get_nc(T)
    in_maps = make_in_maps(x, w_attn, b_attn, w_proj, T)
    res = run_bass_kernel_spmd(nc, in_maps, core_ids=list(range(NCORES)),
                               **spmd_kwargs)
    outs = [np.asarray(r["out"], np.float32) for r in res.results]
    b_eff = (np.asarray(b_proj, np.float32)
             + np.asarray(w_proj, np.float32) @ np.asarray(b_attn, np.float32)[2 * C:])
    full = np.stack(
        [sum(outs[b * CPG:(b + 1) * CPG][1:], outs[b * CPG]) + b_eff
         for b in range(B)]
    ).astype(np.float32)
    return full, res


def kernel(x, w_attn, b_attn, w_proj, b_proj):
    out, _ = run_device(x, w_attn, b_attn, w_proj, b_proj)
    return out


# revision 11
# speedup vs baseline: 2.2244x; 1.0705x over previous
"""Causal self-attention (dense transformer block) on 8 Trainium2 NeuronCores.

Sharding: tensor-parallel over heads x data-parallel over batch.
  - 8 cores = 2 batch groups x 4 cores; each core owns 1 batch element and
    4 of the 16 heads (head_dim 64 -> 256 local channels, 2 "pairs" of heads).
  - Host pre-transposes x and the weight slices; everything ships as bf16
    (3.8e-3 rel err vs the 2e-2 gate) so input DMA is ~6.3 MB/core.
  - Each core computes qkv, causal attention in "S^T" layout (scores[k, q],
    k on partitions), and its partial c_proj; host sums 4 partials per batch.

Math notes:
  - k-bias cancels in softmax; v-bias passes through to a constant output
    offset w_proj @ b_v added on host together with b_proj.
  - Softmax skips max-subtraction: |scores/8| <~ 3 for this distribution.
  - V carries a ones column so softmax denominators fall out of the attV
    matmul (row 64 of the PSUM accumulator).

Schedule (the whole game is keeping PE dense and the HAM clock warm while
ScalarE streams exp):
  - PE warm-up matmuls on a memset tile run while the first DMAs land.
  - x streams per 512-token chunk on two DMA queues (sync + scalar);
    Q/K/V weight column-blocks load Q-first on the gpsimd queue.
  - Scores for the two heads of a pair run CONCURRENTLY as PE row-tiles
    0/64 (tile_position auto-derives from base partitions).
  - exp for both heads of a pair is ONE ScalarE activation over a strided
    [128, 2, w] view of the scores PSUM slab; es is bf16; causal diag
    masking via gpsimd affine_select (off the PE/ACT/DVE critical path).
  - Attention is chunk-major with pairs back-to-back; each pair's softmax
    denominators are reciprocal'd (DVE approx, direct from PSUM row 64),
    partition-broadcast and multiplied into y WHILE the other pair's
    attention runs, so c_proj starts right after the second pair finishes.
  - Deferred K/V projections and split c_proj t-tiles are emitted INSIDE
    the late attention kt-loops as PE filler, since exp (ScalarE) is the
    per-kt critical stage there.
  - Output partials are bf16 (host sums in fp32): halves the output DMA
    and the PSUM->SBUF copies.
"""

import numpy as np
from contextlib import ExitStack

import ml_dtypes

import concourse.bass as bass
import concourse.tile as tile
from concourse import bacc, mybir
from concourse.bass_utils import run_bass_kernel_spmd

FP32 = mybir.dt.float32
BF16 = mybir.dt.bfloat16
AF = mybir.ActivationFunctionType

B, T_FULL, C = 2, 2048, 1024
H, D = 16, 64
NCORES = 8
CPG = 4          # cores per batch group
HPC = H // CPG   # heads per core = 4
HL = HPC * D     # local channels = 256
NP = 2           # head pairs per core
CT = C // 128    # contraction tiles = 8


def build_bass(T=T_FULL):
    """Emit the SPMD Bass/Tile program for one core."""
    assert T % 512 == 0
    NCH = T // 512         # 512-token chunks (4)

    nc = bacc.Bacc("TRN2", target_bir_lowering=False, debug=False,
                   num_devices=NCORES)

    xT_d = nc.dram_tensor("xT", [C, T], BF16, kind="ExternalInput")
    wqkvT_d = nc.dram_tensor("wqkvT", [C, 3 * HL], BF16, kind="ExternalInput")
    bq_d = nc.dram_tensor("bq", [HL], FP32, kind="ExternalInput")
    wpT_d = nc.dram_tensor("wpT", [HL, C], BF16, kind="ExternalInput")
    out_d = nc.dram_tensor("out", [T, C], BF16, kind="ExternalOutput")

    with tile.TileContext(nc) as tc, ExitStack() as ctx:
        xt = ctx.enter_context(tc.tile_pool(name="xt", bufs=CT))
        wq = ctx.enter_context(tc.tile_pool(name="wq", bufs=CT))
        qk = ctx.enter_context(tc.tile_pool(name="qk", bufs=2 * NP))
        vv = ctx.enter_context(tc.tile_pool(name="vv", bufs=NCH))
        es = ctx.enter_context(tc.tile_pool(name="es", bufs=4))
        yt = ctx.enter_context(tc.tile_pool(name="yt", bufs=NP))
        wp = ctx.enter_context(tc.tile_pool(name="wp", bufs=NP))
        ob = ctx.enter_context(tc.tile_pool(name="ob", bufs=3))
        dn = ctx.enter_context(tc.tile_pool(name="dn", bufs=2))
        bc = ctx.enter_context(tc.tile_pool(name="bc", bufs=3))
        sc = ctx.enter_context(tc.tile_pool(name="sc", bufs=1))
        # PSUM budget (8 banks): ss = 2 x [128,1024] slabs (4 banks) for
        # qkv projections + scores; py = 3 x [65,512] attV accumulators;
        # pp = 1 x [128,512] c_proj slab (own pool so next chunk's scores
        # never wait on the c_proj->normalize chain through the ss ring).
        ss = ctx.enter_context(tc.tile_pool(name="ss", bufs=2, space="PSUM"))
        py = ctx.enter_context(tc.tile_pool(name="py", bufs=3, space="PSUM"))
        pp = ctx.enter_context(tc.tile_pool(name="pp", bufs=1, space="PSUM"))

        # ---- PE warm-up: spin the HAM clock while DMAs land ----
        warm_sb = sc.tile([128, 512], BF16, tag="warm")
        nc.vector.memset(warm_sb, 0.0)
        wslab = ss.tile([128, 1024], FP32, tag="ss", name="wslab")
        for i in range(14):
            nc.tensor.matmul(wslab[:, 0:512], warm_sb[:, 0:128], warm_sb,
                             start=(i == 0), stop=(i == 13))

        # ---- weights + constants ----
        wqs = [wq.tile([128, 3 * HL], BF16, tag="wq", name="wtile")
               for _ in range(CT)]
        for blk in range(3):  # Q cols first so projections start early
            for c in range(CT):
                nc.gpsimd.dma_start(
                    out=wqs[c][:, blk * HL:(blk + 1) * HL],
                    in_=wqkvT_d[c * 128:(c + 1) * 128, blk * HL:(blk + 1) * HL])
        wps = []
        for p in range(NP):
            t_ = wp.tile([128, C], BF16, tag="wp", name="wptile")
            nc.gpsimd.dma_start(out=t_, in_=wpT_d[p * 128:(p + 1) * 128, :])
            wps.append(t_)
        bq_sb = sc.tile([128, NP], FP32, tag="bq")
        nc.sync.dma_start(out=bq_sb, in_=bq_d.ap().rearrange("(j p) -> p j", p=128))

        ones_sb = sc.tile([128, 4 * HPC], BF16, tag="ones")
        nc.gpsimd.memset(ones_sb, 1.0)

        # x streams in per 512-token chunk, split across sync+scalar queues
        xts = [xt.tile([128, T], BF16, tag="xt", name="xtile")
               for _ in range(CT)]
        for tchunk in range(NCH):
            t0 = tchunk * 512
            for c in range(CT):
                eng = nc.sync if (c % 2 == 0 or tchunk >= 2) else nc.scalar
                eng.dma_start(out=xts[c][:, t0:t0 + 512],
                              in_=xT_d[c * 128:(c + 1) * 128, t0:t0 + 512])

        qk_tiles = [qk.tile([128, T], BF16, tag="qk", name="qktile")
                    for _ in range(2 * NP)]          # Q0, Q1, K0, K1
        yts = [yt.tile([128, T], BF16, tag="yt", name="ytile")
               for _ in range(NP)]
        vts = []
        for g in range(NCH):
            vt = vv.tile([128, 4, HPC, D + 1], BF16, tag="vv", name="vtile")
            nc.vector.tensor_copy(
                vt[:, :, :, D],
                ones_sb.rearrange("p (a h) -> p a h", a=4),
            )
            vts.append(vt)

        def emit_q(tchunk):
            t0 = tchunk * 512
            slab = ss.tile([128, 1024], FP32, tag="ss", name="pq")
            for p in range(NP):
                for c in range(CT):
                    nc.tensor.matmul(
                        slab[:, p * 512:p * 512 + 512],
                        wqs[c][:, p * 128:p * 128 + 128],
                        xts[c][:, t0:t0 + 512],
                        start=(c == 0), stop=(c == CT - 1),
                    )
            for p in range(NP):
                nc.vector.tensor_scalar_add(
                    qk_tiles[p][:, t0:t0 + 512],
                    slab[:, p * 512:p * 512 + 512], bq_sb[:, p:p + 1])

        def emit_kv(tchunk):
            t0 = tchunk * 512
            slab = ss.tile([128, 1024], FP32, tag="ss", name="pk")
            for p in range(NP):
                for c in range(CT):
                    nc.tensor.matmul(
                        slab[:, p * 512:p * 512 + 512],
                        wqs[c][:, HL + p * 128:HL + p * 128 + 128],
                        xts[c][:, t0:t0 + 512],
                        start=(c == 0), stop=(c == CT - 1),
                    )
            for p in range(NP):
                nc.vector.tensor_copy(
                    qk_tiles[2 + p][:, t0:t0 + 512],
                    slab[:, p * 512:p * 512 + 512])
            slab = ss.tile([128, 1024], FP32, tag="ss", name="pv")
            for tt4 in range(4):
                tt = 4 * tchunk + tt4
                for c in range(CT):
                    nc.tensor.matmul(
                        slab[:, tt4 * 256:tt4 * 256 + 256],
                        xts[c][:, tt * 128:(tt + 1) * 128],
                        wqs[c][:, 2 * HL:3 * HL],
                        start=(c == 0), stop=(c == CT - 1),
                    )
            nc.vector.tensor_copy(
                vts[tchunk][:, :, :, 0:D],
                slab.rearrange("p (a h d) -> p a h d", a=4, h=HPC),
            )

        def emit_attn(p, cg, den_t, fillers=()):
            """Attention for head pair p on q-chunk cg.

            fillers: list of (after_kt, fn) emission hooks for PE filler work.
            Writes unnormalized y into yts and 1/denominator into den_t
            (partitions 0 / 32 for h0 / h1).
            """
            q_t, k_t = qk_tiles[p], qk_tiles[2 + p]
            nkt = 4 * cg + 4
            accs = [py.tile([65, 512], FP32, tag="py", name="acc")
                    for _ in range(2)]
            fillers = list(fillers)
            for kt in range(nkt):
                while fillers and fillers[0][0] <= kt:
                    fillers.pop(0)[1]()
                qa = max(0, kt * 128 - cg * 512)   # local start within chunk
                w = 512 - qa
                q0 = cg * 512 + qa
                slab = ss.tile([128, 1024], FP32, tag="ss", name="pst")
                for h01 in range(2):
                    hb = 64 * h01
                    nc.tensor.matmul(
                        slab[:, h01 * 512:h01 * 512 + w],
                        k_t[hb:hb + 64, kt * 128:(kt + 1) * 128],
                        q_t[hb:hb + 64, q0:q0 + w],
                        start=True, stop=True,
                    )
                es_t = es.tile([128, 2, 512], BF16, tag="es", name="estile")
                nc.scalar.activation(
                    es_t[:, :, 0:w],
                    slab.rearrange("x (h q) -> x h q", h=2)[:, :, 0:w],
                    AF.Exp, scale=0.125,
                )
                if kt >= 4 * cg:  # diagonal block: zero k > q (gpsimd, SBUF)
                    for h01 in range(2):
                        nc.gpsimd.affine_select(
                            out=es_t[:, h01, 0:128], in_=es_t[:, h01, 0:128],
                            compare_op=mybir.AluOpType.is_ge,
                            fill=0.0, base=0,
                            pattern=[[1, 128]], channel_multiplier=-1,
                        )
                for h01 in range(2):
                    nc.tensor.matmul(
                        accs[h01][:, qa:512],
                        vts[kt // 4][:, kt % 4, 2 * p + h01, :],
                        es_t[:, h01, 0:w],
                        start=(kt == 0), stop=(kt == nkt - 1),
                    )
            for f in fillers:
                f[1]()
            cs = slice(cg * 512, cg * 512 + 512)
            for h01 in range(2):
                nc.vector.tensor_copy(
                    yts[p][64 * h01:64 * h01 + 64, cs], accs[h01][0:64, :])
                nc.vector.tensor_copy(
                    den_t[32 * h01:32 * h01 + 1, :], accs[h01][64:65, :])
            nc.vector.reciprocal_approx_fast(den_t[0:33, :], den_t[0:33, :])

        def emit_norm(p, cg, den_t, den_b):
            """Broadcast 1/den and scale this pair's y in place (bf16)."""
            cs = slice(cg * 512, cg * 512 + 512)
            nc.vector.tensor_copy(den_b, den_t[0:33, :])  # fp32 -> bf16
            for h01 in range(2):
                rr = bc.tile([1, 512], BF16, tag="rr", name="rrow")
                nc.sync.dma_start(out=rr, in_=den_b[32 * h01:32 * h01 + 1, :])
                bc_t = bc.tile([128, 512], BF16, tag="bc", name="bct")
                nc.gpsimd.partition_broadcast(bc_t, rr)
                dst = yts[p][64 * h01:64 * h01 + 64, cs]
                nc.vector.tensor_mul(dst, dst, bc_t[64 * h01:64 * h01 + 64, :])

        def emit_cproj(tts, pool=None):
            for tt in tts:
                ot = ob.tile([128, C], BF16, tag="ob", name="otile")
                if pool is None:
                    for s01 in range(2):
                        po = pp.tile([128, 512], FP32, tag="pp", name="po")
                        for p in range(NP):
                            nc.tensor.matmul(
                                po,
                                yts[p][:, tt * 128:(tt + 1) * 128],
                                wps[p][:, s01 * 512:(s01 + 1) * 512],
                                start=(p == 0), stop=(p == NP - 1),
                            )
                        nc.vector.tensor_copy(ot[:, s01 * 512:(s01 + 1) * 512], po)
                else:  # tail: scores are done, reuse the wide ss slabs
                    po = ss.tile([128, 1024], FP32, tag="ss", name="po")
                    for s01 in range(2):
                        for p in range(NP):
                            nc.tensor.matmul(
                                po[:, s01 * 512:(s01 + 1) * 512],
                                yts[p][:, tt * 128:(tt + 1) * 128],
                                wps[p][:, s01 * 512:(s01 + 1) * 512],
                                start=(p == 0), stop=(p == NP - 1),
                            )
                    nc.vector.tensor_copy(ot, po)
                nc.sync.dma_start(out=out_d[tt * 128:(tt + 1) * 128, :], in_=ot)

        # ---- pipelined emission ----
        def den_tiles():
            t_ = dn.tile([128, 512], FP32, tag="dn", name="dent")
            b_ = dn.tile([33, 512], BF16, tag="dnb", name="denb")
            return t_, b_

        emit_q(0); emit_kv(0)
        for cg in range(NCH):
            dts = [den_tiles() for _ in range(NP)]
            if cg == 0:
                emit_attn(0, cg, dts[0][0])
                emit_norm(0, cg, *dts[0])
                emit_q(1); emit_kv(1)
                emit_attn(1, cg, dts[1][0])
                emit_norm(1, cg, *dts[1])
            elif cg == 1:
                emit_attn(0, cg, dts[0][0],
                          fillers=[(2, lambda: emit_cproj(range(0, 4)))])
                emit_norm(0, cg, *dts[0])
                emit_q(2); emit_kv(2)
                emit_attn(1, cg, dts[1][0])
                emit_norm(1, cg, *dts[1])
            elif cg == 2:
                emit_attn(0, cg, dts[0][0],
                          fillers=[(2, lambda: emit_cproj(range(4, 8)))])
                emit_norm(0, cg, *dts[0])
                emit_attn(1, cg, dts[1][0],
                          fillers=[(4, lambda: (emit_q(3), emit_kv(3)))])
                emit_norm(1, cg, *dts[1])
            else:
                emit_attn(0, cg, dts[0][0],
                          fillers=[(2, lambda: emit_cproj(range(8, 10))),
                                   (8, lambda: emit_cproj(range(10, 12)))])
                emit_norm(0, cg, *dts[0])
                emit_attn(1, cg, dts[1][0])
                emit_norm(1, cg, *dts[1])
                emit_cproj(range(12, 16), pool="ss")

    nc.compile()
    return nc


_NC_CACHE = {}


def _get_nc(T=T_FULL):
    if T not in _NC_CACHE:
        _NC_CACHE[T] = build_bass(T)
    return _NC_CACHE[T]


def make_in_maps(x, w_attn, b_attn, w_proj, T=T_FULL):
    bf = ml_dtypes.bfloat16
    x = np.ascontiguousarray(np.asarray(x, np.float32))
    w_attn = np.asarray(w_attn, np.float32)
    b_attn = np.asarray(b_attn, np.float32)
    w_proj = np.asarray(w_proj, np.float32)
    xTs = [np.ascontiguousarray(x[b].T.astype(bf)) for b in range(x.shape[0])]
    in_maps = []
    for core in range(NCORES):
        b, j = core // CPG, core % CPG
        r0 = j * HL
        wq_s = w_attn[r0:r0 + HL]
        wk_s = w_attn[C + r0:C + r0 + HL]
        wv_s = w_attn[2 * C + r0:2 * C + r0 + HL]
        in_maps.append({
            "xT": xTs[b],
            "wqkvT": np.ascontiguousarray(
                np.concatenate([wq_s, wk_s, wv_s], axis=0).T.astype(bf)),
            "bq": np.ascontiguousarray(b_attn[r0:r0 + HL]),
            "wpT": np.ascontiguousarray(w_proj[:, r0:r0 + HL].T.astype(bf)),
        })
    return in_maps


def run_device(x, w_attn, b_attn, w_proj, b_proj, T=T_FULL, **spmd_kwargs):
    nc = _get_nc(T)
    in_maps = make_in_maps(x, w_attn, b_attn, w_proj, T)
    res = run_bass_kernel_spmd(nc, in_maps, core_ids=list(range(NCORES)),
                               **spmd_kwargs)
    outs = [np.asarray(r["out"], np.float32) for r in res.results]
    b_eff = (np.asarray(b_proj, np.float32)
             + np.asarray(w_proj, np.float32) @ np.asarray(b_attn, np.float32)[2 * C:])
    full = np.stack(
        [sum(outs[b * CPG:(b + 1) * CPG][1:], outs[b * CPG]) + b_eff
         for b in range(B)]
    ).astype(np.float32)
    return full, res


def kernel(x, w_attn, b_attn, w_proj, b_proj):
    out, _ = run_device(x, w_attn, b_attn, w_proj, b_proj)
    return out
